# revision 11
# baseline (speedup 1.0000x reference)
"""Trainium2 Bass kernel for nn_DNN_24464133718540 (embedding_lookup).

Reference computation:
    emb[b,f]  = tables[f, src[b,f]]            # [B, 45, 256] gather
    h         = emb @ W1 + b1                  # [B, 45, 32]
    out[b,f]  = h @ W2 + b2                    # [B, 45, 1]
    result[b] = sum_f out[b,f]                 # [B, 1]

The MLP is linear (no activation), so with w = W1 @ W2 ([256]) and
c = b1 @ W2 + b2 (scalar):
    result[b] = sum_f tables[f, src[b,f]] . w  +  45 * c

i.e. the whole network collapses to one score per table row,
score[f, v] = tables[f, v] . w, and per-sample sums of 45 scores.

Device strategy (SPMD over 8 cores):
  All 45 tables' rows are flat-packed into one 450000-column score
  space; each core owns exactly 56250 columns (27 units of 2048 cols +
  one 954-col tail unit, padded to 1024) -- perfectly balanced, no
  dummy work.  Units are staged host-side TRANSPOSED as [d=256, v]
  fp8-e4m3 (x16 scaled; the measured end-to-end error is ~1e-3 against
  a 2e-2 budget), so the per-row dot product with w becomes a TensorE
  DoubleRow matmul: 2 fp8 weights per PE cell contract all 256 d in a
  single pass over the columns.
  Per unit: one DMA streams the [128, 2, v] K-paired tile; 4 (or 2)
  DoubleRow matmuls accumulate scores into PSUM [128, v]; ScalarE/DVE
  (alternating) copy score row 0 to SBUF and an 8 KB DMA writes it out.
  All 115 MB of fp8 table bytes and all 115M MACs stay on device; the
  per-core output is its 225 KB score-table shard.
Host: index-select of the per-(b, f) scores + bincount reduction to
  [B, 1] (same order of work as the baseline's host-side partial-sum
  reduction), + 45*c, undoing the 16*64 fp8 scaling.
"""

import numpy as np

B, F, V, D, H = 16384, 45, 10000, 256, 32
NCORES = 8
GTOT = F * V             # 450000 flat score columns
PC = GTOT // NCORES      # 56250 columns per core
CH = 2048                # columns per full unit
NU = 27                  # full units per core
TAIL = PC - NU * CH      # 954 real tail columns
TAILP = 1024             # padded tail width
PCP = NU * CH + TAILP    # 56320 staged columns per core
TSC = 16.0               # table fp8 scale
WSC = 64.0               # w fp8 scale

_COMPILED = {}


def _build_program():
    import concourse.bacc as bacc
    import concourse.tile as tile
    from concourse import mybir

    f32 = mybir.dt.float32
    f8 = mybir.dt.float8e4

    nc = bacc.Bacc("TRN2", target_bir_lowering=False, debug=False,
                   num_devices=NCORES)

    tabc_d = nc.dram_tensor("tabc", [D, PCP], f8, kind="ExternalInput")
    w_d = nc.dram_tensor("wT", [128, 256], f8, kind="ExternalInput")
    out_d = nc.dram_tensor("out_part", [1, PCP], f32, kind="ExternalOutput")

    # stream spans: 1 MB (2-unit) DMAs for peak engine rate; two small first
    # spans prime the pipeline quickly, the padded tail unit rides alone
    spans = [512, 1536] + [2 * CH] * 13 + [TAILP]

    with tile.TileContext(nc) as tc:
        with (
            tc.tile_pool(name="const", bufs=1) as const_pool,
            tc.tile_pool(name="stream", bufs=5) as stream_pool,
            tc.tile_pool(name="rep", bufs=4) as rep_pool,
            tc.tile_pool(name="ps", bufs=2, space="PSUM") as psum_pool,
        ):
            w_t = const_pool.tile([128, 256], f8, tag="w")
            nc.sync.dma_start(w_t[:], w_d.ap())
            w3 = w_t[:].rearrange("p (j m) -> p j m", j=2)

            lo = 0
            u = 0
            for span in spans:
                # K-paired stream tile: st[p, j, i] = tab[j*128 + p, lo + i]
                st = stream_pool.tile([128, 2 * span], f8, tag="st")
                nc.sync.dma_start(
                    st[:].rearrange("p (j i) -> p j i", j=2),
                    tabc_d.ap()[:, lo:lo + span].rearrange(
                        "(j p) i -> p j i", j=2))
                st3 = st[:].rearrange("p (j i) -> p j i", j=2)

                base = 0
                while base < span:
                    size = min(CH, span - base)
                    ps = psum_pool.tile([128, size], f32, tag="ps")
                    for q in range(size // 512):
                        nc.tensor.matmul(
                            ps[:, q * 512:(q + 1) * 512],
                            w3,
                            st3[:, :, base + q * 512:base + (q + 1) * 512],
                            start=True, stop=True,
                            perf_mode=mybir.MatmulPerfMode.DoubleRow)

                    # score row is replicated in every PSUM partition; copy
                    # row 0 to SBUF (halves on ScalarE + DVE) and DMA it out.
                    rep = rep_pool.tile([128, CH], f32, tag="rep")
                    hh = size // 2
                    nc.scalar.copy(rep[0:1, :hh], ps[0:1, :hh])
                    nc.vector.tensor_copy(rep[0:1, hh:size], ps[0:1, hh:])
                    nc.gpsimd.dma_start(
                        out_d.ap()[:, lo + base:lo + base + size],
                        rep[0:1, :size])
                    base += size
                    u += 1
                lo += span

    nc.compile()
    return nc


def _get_program():
    if "nc" not in _COMPILED:
        _COMPILED["nc"] = _build_program()
    return _COMPILED["nc"]


def kernel(src, tables, W1, b1, W2, b2, _trace=False, _trace_cores=None,
           _tmpdir=None):
    import ml_dtypes
    from concourse.bass_utils import run_bass_kernel_spmd

    f8np = ml_dtypes.float8_e4m3

    src_i = np.asarray(src).astype(np.int64)
    tables = np.asarray(tables, dtype=np.float32)
    W1 = np.asarray(W1, dtype=np.float32)
    b1 = np.asarray(b1, dtype=np.float32)
    W2 = np.asarray(W2, dtype=np.float32)
    b2 = np.asarray(b2, dtype=np.float32)

    w = (W1 @ W2).reshape(D)                      # [256]
    c = float(b1 @ W2[:, 0] + b2[0])              # scalar per feature

    # flat-packed transposed fp8 tables: [256, 450000], x16 scaled
    tabT = np.concatenate([tables[f].T for f in range(F)], axis=1)
    tab8 = (tabT * TSC).astype(f8np)              # [256, 450000]

    wT = np.zeros((128, 256), dtype=np.float32)
    wT[:, :128] = w[:128, None]
    wT[:, 128:] = w[128:, None]
    w8 = (wT * WSC).astype(f8np)

    in_maps = []
    for cidx in range(NCORES):
        tabc = np.zeros((D, PCP), dtype=f8np)
        tabc[:, :PC] = tab8[:, cidx * PC:(cidx + 1) * PC]
        in_maps.append({"tabc": tabc, "wT": w8})

    nc = _get_program()
    kw = {}
    if _trace:
        kw = {"trace": True, "trace_cores": _trace_cores or [0],
              "tmpdir": _tmpdir}
    res = run_bass_kernel_spmd(nc, in_maps, core_ids=list(range(NCORES)), **kw)
    _COMPILED["last_results"] = res

    # unshard: core c's out[:PC] are flat score columns [c*PC, (c+1)*PC)
    scores_flat = np.concatenate(
        [res.results[cidx]["out_part"].ravel()[:PC]
         for cidx in range(NCORES)]).astype(np.float64) / (TSC * WSC)
    ptr = (np.arange(F)[None, :] * V + src_i).ravel()
    bb = np.repeat(np.arange(B), F)
    total = np.bincount(bb, weights=scores_flat[ptr], minlength=B)
    return (total + F * c).astype(np.float32).reshape(B, 1)
